# revision 11
# baseline (speedup 1.0000x reference)
"""Trainium2 Bass kernel for nn_Decoder: 32-step LSTMCell decoder + 32000-way
vocab projection, SPMD across 8 NeuronCores.

Sharding: LSTM recurrence replicated on all cores (it is sequential and small);
vocab projection tensor-parallel column-sharded 4000 cols/core. No collectives;
host gathers logit slices and computes argmax (with exact fp64 refinement of
top-k candidates from the kernel's fp32 hx output, so predictions match an
fp32 reference even though the vocab matmul runs in bf16).

Self-contained: only needs numpy/ml_dtypes and /opt/trn_rl_repo (concourse).
"""
import sys

sys.path.insert(0, "/opt/trn_rl_repo")

import numpy as np
import ml_dtypes

import concourse.bass as bass
import concourse.tile as tile
from concourse import bacc, mybir
from concourse.bass_utils import run_bass_kernel_spmd
from concourse.masks import make_identity

B, L, D, H, V = 64, 32, 512, 1024, 32000
NCORES = 8
VS = V // NCORES            # 4000 vocab cols per core
G4 = 4 * H                  # 4096 gate width
KD, KH = D // 128, H // 128  # k-tiles: 4 (input dim), 8 (hidden dim)
R = L * B                   # 2048 output rows, r = t*64 + b
GCHUNK = 512                # gates N-chunk
VCHUNK = 500                # vocab N-chunk
NVC = VS // VCHUNK          # 8 vocab chunks
WIN = 17                    # hxT ring slots; 2mt+1 is odd so vocab
                            # M-tiles never straddle the ring wrap
NMT = R // 128              # 16 vocab M-tiles (2 steps each)
MT_BATCH = 4                # vocab M-tiles per batch (8 steps)

TERMS = 3                   # 1 = bf16 single; 3 = bf16 hi/lo 3-term split
F32, F16, BF16 = mybir.dt.float32, mybir.dt.float16, mybir.dt.bfloat16
SIGMOID = mybir.ActivationFunctionType.Sigmoid
TANH = mybir.ActivationFunctionType.Tanh


def build_nc(terms=TERMS):
    nc = bacc.Bacc("TRN2", target_bir_lowering=False, debug=False,
                   num_devices=NCORES)

    def din(name, shape, dt):
        return nc.dram_tensor(name, shape, dt, kind="ExternalInput").ap()

    xT_hi = din("xT_hi", [KD, 128, R], BF16)
    hsT = din("hsT", [KD, 128, B], F32)
    wp = din("wp", [KD, 128, H], F32)
    bp = din("bp", [H], F32)
    wih_hi = din("wih_hi", [KD, 128, G4], BF16)
    whh_hi = din("whh_hi", [KH, 128, G4], BF16)
    bg = din("bg", [G4], F32)
    wv = din("wv", [KH, 128, VS], BF16)
    if terms == 3:
        xT_lo = din("xT_lo", [KD, 128, R], BF16)
        wih_lo = din("wih_lo", [KD, 128, G4], BF16)
        whh_lo = din("whh_lo", [KH, 128, G4], BF16)

    logits = nc.dram_tensor("logits", [R, VS], F16, kind="ExternalOutput").ap()
    hx_out = nc.dram_tensor("hx_out", [L, B, H], F32, kind="ExternalOutput").ap()
    xw = nc.dram_tensor("xw", [R, G4], F32).ap()  # internal scratch

    def bcast(src, n):
        # broadcast a [N] dram vector across n partitions
        return bass.AP(tensor=src.tensor, offset=src.offset,
                       ap=[[0, n], *src.ap])

    with tile.TileContext(nc) as tc:
        with (
            tc.tile_pool(name="consts", bufs=1) as consts,
            tc.tile_pool(name="whhp", bufs=1) as whhp,
            tc.tile_pool(name="hist", bufs=1) as histp,
            tc.tile_pool(name="state", bufs=1) as statep,
            tc.tile_pool(name="gps", bufs=2, space="PSUM") as gps,
            tc.tile_pool(name="vps", bufs=2, space="PSUM") as vps,
            tc.tile_pool(name="tps", bufs=2, space="PSUM") as tps,
        ):
            ident = consts.tile([64, 64], F32)
            make_identity(nc, ident)

            # ---- persistent tiles ----
            whh_sb = [whhp.tile([128, G4], BF16, tag=f"whhh{k}", name=f"whhh{k}") for k in range(KH)]
            for k in range(KH):
                nc.sync.dma_start(whh_sb[k][:], whh_hi[k])

            # hxT ring: slot s holds transpose(h_s) bf16 (+ lo for recurrence)
            hxT = [histp.tile([128, WIN * B], BF16, tag=f"hxT{k}", name=f"hxT{k}") for k in range(KH)]
            if terms == 3:
                hxTlo = [histp.tile([128, 2 * B], BF16, tag=f"hxTlo{k}", name=f"hxTlo{k}")
                         for k in range(KH)]

            hx_sb = statep.tile([64, H], F32)   # hidden state h_t

            # ---- phase 1+2: hx0 and XW precompute (scoped pools) ----
            with (
                tc.tile_pool(name="ph2", bufs=1) as ph2,
                tc.tile_pool(name="ph2st", bufs=3) as ph2st,
            ):
                # hx0 = hs_last @ W_p.T + b_p
                hs_sb = [ph2st.tile([128, B], F32, tag=f"hs{k}", name=f"hs{k}",
                                     bufs=1) for k in range(KD)]
                wp_sb = [ph2st.tile([128, H], F32, tag=f"wpk{k}", name=f"wpk{k}",
                                    bufs=1) for k in range(KD)]
                for k in range(KD):
                    nc.sync.dma_start(hs_sb[k][:], hsT[k])
                    nc.sync.dma_start(wp_sb[k][:], wp[k])
                bp_bc = ph2st.tile([64, H], F32, tag="bpb", bufs=1)
                nc.gpsimd.dma_start(bp_bc[:], bcast(bp, 64))
                for c in range(H // GCHUNK):
                    ps = gps.tile([64, GCHUNK], F32, tag="g")
                    cs = slice(c * GCHUNK, (c + 1) * GCHUNK)
                    for k in range(KD):
                        nc.tensor.matmul(ps[:], hs_sb[k][:], wp_sb[k][:, cs],
                                         start=(k == 0), stop=(k == KD - 1))
                    nc.vector.tensor_add(hx_sb[:, cs], ps[:], bp_bc[:, cs])

                # XW = x @ W_ih.T + (b_ih + b_hh), rows r = t*64+b
                bg_bc = ph2.tile([128, G4], F32)
                nc.gpsimd.dma_start(bg_bc[:], bcast(bg, 128))
                wih_sb = [ph2.tile([128, G4], BF16, tag=f"wi{k}", name=f"wi{k}") for k in range(KD)]
                for k in range(KD):
                    nc.sync.dma_start(wih_sb[k][:], wih_hi[k])
                if terms == 3:
                    wil_sb = [ph2.tile([128, G4], BF16, tag=f"wil{k}", name=f"wil{k}")
                              for k in range(KD)]
                    for k in range(KD):
                        nc.sync.dma_start(wil_sb[k][:], wih_lo[k])
                nmm = (3 * KD) if terms == 3 else KD
                for mt in range(NMT):
                    ms = slice(mt * 128, (mt + 1) * 128)
                    xts = [ph2st.tile([128, 128], BF16, tag=f"xts{k}",
                                      name=f"xts{k}", bufs=2) for k in range(KD)]
                    for k in range(KD):
                        nc.sync.dma_start(xts[k][:], xT_hi[k][:, ms])
                    xw_terms = [(xts, wih_sb)]
                    if terms == 3:
                        xtls = [ph2st.tile([128, 128], BF16, tag=f"xtls{k}",
                                           name=f"xtls{k}", bufs=2)
                                for k in range(KD)]
                        for k in range(KD):
                            nc.sync.dma_start(xtls[k][:], xT_lo[k][:, ms])
                        xw_terms += [(xtls, wih_sb), (xts, wil_sb)]
                    for c in range(G4 // GCHUNK):
                        cs = slice(c * GCHUNK, (c + 1) * GCHUNK)
                        ps = gps.tile([128, GCHUNK], F32, tag="g")
                        j = 0
                        for lhs, rhs in xw_terms:
                            for k in range(KD):
                                nc.tensor.matmul(ps[:], lhs[k][:], rhs[k][:, cs],
                                                 start=(j == 0), stop=(j == nmm - 1))
                                j += 1
                        st = ph2st.tile([128, GCHUNK], F32, tag="xwst")
                        nc.vector.tensor_add(st[:], ps[:], bg_bc[:, cs])
                        nc.sync.dma_start(xw[ms, cs], st[:])

            # ---- loop-phase pools (reuse phase-2 address space) ----
            import contextlib
            _loop = contextlib.ExitStack()
            whlp = _loop.enter_context(tc.tile_pool(name="whlp", bufs=1))
            cellp = _loop.enter_context(tc.tile_pool(name="cell", bufs=1))
            xwbufp = _loop.enter_context(tc.tile_pool(name="xwbuf", bufs=1))
            gbufp = _loop.enter_context(tc.tile_pool(name="gbuf", bufs=2))
            wvsp = _loop.enter_context(tc.tile_pool(name="wvs", bufs=2))
            lstagep = _loop.enter_context(tc.tile_pool(name="lstage", bufs=2))

            if terms == 3:
                whl_sb = [whlp.tile([128, G4], BF16, tag=f"whhl{k}", name=f"whhl{k}")
                          for k in range(KH)]
                for k in range(KH):
                    nc.sync.dma_start(whl_sb[k][:], whh_lo[k])

            xwt = xwbufp.tile([128, G4], F32)   # xw(t) parity-packed
            cx_sb = cellp.tile([64, H], F32, tag="cx")
            nc.vector.memset(cx_sb[:], 0.0)
            sig_i = cellp.tile([64, H], F32, tag="si")   # later reused as i*g~
            tanh_g = cellp.tile([64, H], F32, tag="tg")  # later reused as tanh(cx)
            sig_o = cellp.tile([64, H], F32, tag="so")

            # ---- transpose h_t -> hxT ring slot (bf16 hi [+ lo]) ----
            def transpose_h(t):
                s = t % WIN
                for k in range(KH):
                    pst = tps.tile([128, 64], F32, tag="tp")
                    nc.tensor.transpose(pst[:], hx_sb[:, k * 128:(k + 1) * 128],
                                        ident[:])
                    dst = hxT[k][:, s * B:(s + 1) * B]
                    nc.vector.tensor_copy(dst, pst[:])
                    if terms == 3:
                        dlo = hxTlo[k][:, (t % 2) * B:(t % 2 + 1) * B]
                        nc.vector.tensor_sub(dlo, pst[:], dst)

            # ---- vocab job machinery ----
            wv_cur = {}

            def stream_wv(c):
                tiles = [wvsp.tile([128, VCHUNK], BF16, tag=f"wv{k}", name=f"wvs{k}")
                         for k in range(KH)]
                cs = slice(c * VCHUNK, (c + 1) * VCHUNK)
                for k in range(KH):
                    nc.sync.dma_start(tiles[k][:], wv[k][:, cs])
                return tiles

            def vocab_job(mt, c, wv_tiles):
                t0 = 2 * mt + 1          # rows mt*128.. use h_{t0}, h_{t0+1}
                s0, s1 = t0 % WIN, (t0 + 1) % WIN
                if s1 == s0 + 1:
                    ps = vps.tile([128, VCHUNK], F32, tag="vp")
                    lh = [hxT[k][:, s0 * B:(s0 + 2) * B] for k in range(KH)]
                    for k in range(KH):
                        nc.tensor.matmul(ps[:], lh[k], wv_tiles[k][:],
                                         start=(k == 0), stop=(k == KH - 1))
                    st = lstagep.tile([128, VCHUNK], F16, tag="lg")
                    nc.scalar.copy(st[:], ps[:])
                if s1 != s0 + 1:  # ring wrap: two M=64 halves, separate psums
                    st = lstagep.tile([128, VCHUNK], F16, tag="lg")
                    for half, s in ((0, s0), (1, s1)):
                        psh = vps.tile([64, VCHUNK], F32, tag=f"vpw{half}",
                                       name=f"vpw{half}", bufs=1)
                        for k in range(KH):
                            nc.tensor.matmul(psh[:], hxT[k][:, s * B:(s + 1) * B],
                                             wv_tiles[k][:],
                                             start=(k == 0), stop=(k == KH - 1))
                        nc.scalar.copy(st[half * 64:(half + 1) * 64, :], psh[:])
                nc.sync.dma_start(
                    logits[mt * 128:(mt + 1) * 128, c * VCHUNK:(c + 1) * VCHUNK],
                    st[:])

            vocab_q = []   # list of (mt, c) in stream order (c outer within batch)

            def emit_vocab(n):
                nonlocal wv_cur
                for _ in range(n):
                    if not vocab_q:
                        return
                    mt, c = vocab_q.pop(0)
                    if c not in wv_cur:
                        wv_cur = {c: stream_wv(c)}
                    vocab_job(mt, c, wv_cur[c])

            # ---- main recurrence ----
            transpose_h(0)
            # prefetch xw row-block for t=0 into parity half 0
            nc.sync.dma_start(xwt[0:64, :], xw[0:64, :])
            for t in range(L):
                par, nxt = (t % 2) * 64, ((t + 1) % 2) * 64
                if t + 1 < L:
                    nc.sync.dma_start(xwt[nxt:nxt + 64, :],
                                      xw[(t + 1) * 64:(t + 2) * 64, :])
                st_slot = t % WIN
                gate_t = [(hxT[k][:, st_slot * B:(st_slot + 1) * B], whh_sb[k])
                          for k in range(KH)]
                if terms == 3:
                    gate_t += [(hxT[k][:, st_slot * B:(st_slot + 1) * B], whl_sb[k])
                               for k in range(KH)]
                    gate_t += [(hxTlo[k][:, (t % 2) * B:(t % 2 + 1) * B], whh_sb[k])
                               for k in range(KH)]
                gparts = (sig_i, None, tanh_g, sig_o)
                gfuncs = (SIGMOID, SIGMOID, TANH, SIGMOID)
                for c in range(G4 // GCHUNK):
                    cs = slice(c * GCHUNK, (c + 1) * GCHUNK)
                    ps = gps.tile([64, GCHUNK], F32, tag="g")
                    for j, (lh, rh) in enumerate(gate_t):
                        nc.tensor.matmul(ps[:], lh, rh[:, cs],
                                         start=(j == 0), stop=(j == len(gate_t) - 1))
                    gb = gbufp.tile([64, GCHUNK], F32, tag="gb")
                    nc.vector.tensor_add(gb[:], ps[:], xwt[par:par + 64, cs])
                    # chunk c covers gate part c//2, half c%2
                    part, half = gparts[c // 2], c % 2
                    hs = slice(half * GCHUNK, (half + 1) * GCHUNK)
                    if part is None:  # f gate: fold sigmoid(f)*cx chunk-wise
                        nc.scalar.activation(gb[:], gb[:], SIGMOID)
                        nc.vector.tensor_mul(cx_sb[:, hs], cx_sb[:, hs], gb[:])
                    else:
                        nc.scalar.activation(part[:, hs], gb[:], gfuncs[c // 2])
                emit_vocab(2)
                # cell update
                nc.vector.tensor_mul(sig_i[:], sig_i[:], tanh_g[:])     # i*g~
                nc.vector.tensor_add(cx_sb[:], cx_sb[:], sig_i[:])
                nc.scalar.activation(tanh_g[:], cx_sb[:], TANH)          # tanh(cx)
                nc.vector.tensor_mul(hx_sb[:], sig_o[:], tanh_g[:])
                nc.sync.dma_start(hx_out[t], hx_sb[:])
                transpose_h(t + 1)
                emit_vocab(2)
                if t % 8 == 7:
                    b = t // 8
                    vocab_q += [(4 * b + i, c) for c in range(NVC)
                                for i in range(MT_BATCH)]
            while vocab_q:
                emit_vocab(1)
            _loop.close()

    return nc


_CACHED = {}


def get_compiled():
    if "nc" not in _CACHED:
        nc = build_nc()
        nc.compile()
        _CACHED["nc"] = nc
    return _CACHED["nc"]


def _bf16(x):
    return np.asarray(x, np.float32).astype(ml_dtypes.bfloat16)


def _split(x):
    hi = _bf16(x)
    lo = _bf16(np.asarray(x, np.float32) - hi.astype(np.float32))
    return hi, lo


def prep_in_maps(sent_inputs, hidden_state, W_ih, W_hh, b_ih, b_hh,
                 W_p, b_p, W_v, b_v):
    f32 = np.float32
    xT = np.ascontiguousarray(
        np.asarray(sent_inputs, f32).transpose(2, 1, 0)).reshape(KD, 128, R)
    xT_hi, xT_lo = _split(xT)
    hsT = np.ascontiguousarray(
        np.asarray(hidden_state, f32)[:, -1, :].T).reshape(KD, 128, B)
    wp = np.ascontiguousarray(np.asarray(W_p, f32).T).reshape(KD, 128, H)
    wih = np.ascontiguousarray(np.asarray(W_ih, f32).T).reshape(KD, 128, G4)
    wih_hi, wih_lo = _split(wih)
    whh = np.ascontiguousarray(np.asarray(W_hh, f32).T).reshape(KH, 128, G4)
    whh_hi, whh_lo = _split(whh)
    bg = (np.asarray(b_ih, f32) + np.asarray(b_hh, f32)).astype(f32)
    wvT = np.ascontiguousarray(np.asarray(W_v, f32).T)  # [H, V]
    base = dict(xT_hi=xT_hi, hsT=hsT, wp=wp, bp=np.asarray(b_p, f32),
                wih_hi=wih_hi, bg=bg)
    if TERMS == 3:
        base.update(xT_lo=xT_lo, wih_lo=wih_lo, whh_lo=whh_lo)
    in_maps = []
    for c in range(NCORES):
        m = dict(base)
        m["whh_hi"] = whh_hi
        sl = wvT[:, c * VS:(c + 1) * VS]
        m["wv"] = np.ascontiguousarray(sl).reshape(KH, 128, VS).astype(
            ml_dtypes.bfloat16)
        in_maps.append(m)
    return in_maps


def postprocess(results, W_v, b_v):
    f32 = np.float32
    logits = np.concatenate(
        [results[c]["logits"].astype(f32) for c in range(NCORES)], axis=1)
    logits += np.asarray(b_v, f32)[None, :]
    hx = results[0]["hx_out"]                      # [L, B, H] fp32
    # logits rows r = t*64+b -> [B, L, V]
    logits_blv = np.ascontiguousarray(
        logits.reshape(L, B, V).transpose(1, 0, 2))
    # predicts: top-k candidates from device logits, exact fp64 recompute
    K = 32
    flat = logits.reshape(R, V)
    cand = np.argpartition(-flat, K, axis=1)[:, :K]
    cand.sort(axis=1)
    Wv64 = np.asarray(W_v, np.float64)
    bv64 = np.asarray(b_v, np.float64)
    hx64 = hx.reshape(R, H).astype(np.float64)
    ex = np.einsum("rh,rkh->rk", hx64, Wv64[cand]) + bv64[cand]
    pick = cand[np.arange(R), np.argmax(ex, axis=1)]
    predicts = pick.reshape(L, B).T.astype(f32)    # [B, L]
    return logits_blv, predicts


def kernel(**inputs):
    nc = get_compiled()
    in_maps = prep_in_maps(**inputs)
    res = run_bass_kernel_spmd(nc, in_maps, core_ids=list(range(NCORES)))
    return postprocess(res.results, inputs["W_v"], inputs["b_v"])


# revision 15
# speedup vs baseline: 1.0007x; 1.0007x over previous
"""Trainium2 Bass kernel for nn_Decoder: 32-step LSTMCell decoder + 32000-way
vocab projection, SPMD across 8 NeuronCores.

Sharding: LSTM recurrence replicated on all cores (it is sequential and small);
vocab projection tensor-parallel column-sharded 4000 cols/core. No collectives;
host gathers logit slices and computes argmax (with exact fp64 refinement of
top-k candidates from the kernel's fp32 hx output, so predictions match an
fp32 reference even though the vocab matmul runs in bf16).

Self-contained: only needs numpy/ml_dtypes and /opt/trn_rl_repo (concourse).
"""
import sys

sys.path.insert(0, "/opt/trn_rl_repo")

import numpy as np
import ml_dtypes

import concourse.bass as bass
import concourse.tile as tile
from concourse import bacc, mybir
from concourse.bass_utils import run_bass_kernel_spmd
from concourse.masks import make_identity

B, L, D, H, V = 64, 32, 512, 1024, 32000
NCORES = 8
VS = V // NCORES            # 4000 vocab cols per core
G4 = 4 * H                  # 4096 gate width
KD, KH = D // 128, H // 128  # k-tiles: 4 (input dim), 8 (hidden dim)
R = L * B                   # 2048 output rows, r = t*64 + b
GCHUNK = 512                # gates N-chunk
VCHUNK = 500                # vocab N-chunk
NVC = VS // VCHUNK          # 8 vocab chunks
WIN = 17                    # hxT ring slots; 2mt+1 is odd so vocab
                            # M-tiles never straddle the ring wrap
NMT = R // 128              # 16 vocab M-tiles (2 steps each)
MT_BATCH = 4                # vocab M-tiles per batch (8 steps)

TERMS = 3                   # 1 = bf16 single; 3 = bf16 hi/lo 3-term split
PACK = True                 # col-tile-pack gate matmuls (2 chains, halves PE time)
F32, F16, BF16 = mybir.dt.float32, mybir.dt.float16, mybir.dt.bfloat16
SIGMOID = mybir.ActivationFunctionType.Sigmoid
TANH = mybir.ActivationFunctionType.Tanh


def build_nc(terms=TERMS, pack=PACK):
    nc = bacc.Bacc("TRN2", target_bir_lowering=False, debug=False,
                   num_devices=NCORES)

    def din(name, shape, dt):
        return nc.dram_tensor(name, shape, dt, kind="ExternalInput").ap()

    xT_hi = din("xT_hi", [KD, 128, R], BF16)
    hsT = din("hsT", [KD, 128, B], F32)
    wp = din("wp", [KD, 128, H], F32)
    bp = din("bp", [H], F32)
    wih_hi = din("wih_hi", [KD, 128, G4], BF16)
    whh_hi = din("whh_hi", [KH, 128, G4], BF16)
    bg = din("bg", [G4], F32)
    wv = din("wv", [KH, 128, VS], BF16)
    if terms == 3:
        xT_lo = din("xT_lo", [KD, 128, R], BF16)
        wih_lo = din("wih_lo", [KD, 128, G4], BF16)
        whh_lo = din("whh_lo", [KH, 128, G4], BF16)

    logits = nc.dram_tensor("logits", [R, VS], F16, kind="ExternalOutput").ap()
    hx_out = nc.dram_tensor("hx_out", [L, B, H], F32, kind="ExternalOutput").ap()
    xw = nc.dram_tensor("xw", [R, G4], F32).ap()  # internal scratch

    def bcast(src, n):
        # broadcast a [N] dram vector across n partitions
        return bass.AP(tensor=src.tensor, offset=src.offset,
                       ap=[[0, n], *src.ap])

    with tile.TileContext(nc) as tc:
        with (
            tc.tile_pool(name="consts", bufs=1) as consts,
            tc.tile_pool(name="whhp", bufs=1) as whhp,
            tc.tile_pool(name="hist", bufs=1) as histp,
            tc.tile_pool(name="state", bufs=1) as statep,
            tc.tile_pool(name="gps", bufs=2, space="PSUM") as gps,
            tc.tile_pool(name="vps", bufs=2, space="PSUM") as vps,
            tc.tile_pool(name="tps", bufs=2, space="PSUM") as tps,
        ):
            ident = consts.tile([64, 64], F32)
            make_identity(nc, ident)

            # ---- persistent tiles ----
            whh_sb = [whhp.tile([128, G4], BF16, tag=f"whhh{k}", name=f"whhh{k}") for k in range(KH)]
            for k in range(KH):
                nc.sync.dma_start(whh_sb[k][:], whh_hi[k])

            # hxT ring: slot s holds transpose(h_s) bf16 (+ lo for recurrence)
            hxT = [histp.tile([128, WIN * B], BF16, tag=f"hxT{k}", name=f"hxT{k}") for k in range(KH)]
            if terms == 3:
                hxTlo = [histp.tile([128, 2 * B], BF16, tag=f"hxTlo{k}", name=f"hxTlo{k}")
                         for k in range(KH)]

            hx_sb = statep.tile([64, H], F32)   # hidden state h_t

            # ---- phase 1+2: hx0 and XW precompute (scoped pools) ----
            with (
                tc.tile_pool(name="ph2", bufs=1) as ph2,
                tc.tile_pool(name="ph2st", bufs=3) as ph2st,
            ):
                # hx0 = hs_last @ W_p.T + b_p
                hs_sb = [ph2st.tile([128, B], F32, tag=f"hs{k}", name=f"hs{k}",
                                     bufs=1) for k in range(KD)]
                wp_sb = [ph2st.tile([128, H], F32, tag=f"wpk{k}", name=f"wpk{k}",
                                    bufs=1) for k in range(KD)]
                for k in range(KD):
                    nc.sync.dma_start(hs_sb[k][:], hsT[k])
                    nc.sync.dma_start(wp_sb[k][:], wp[k])
                bp_bc = ph2st.tile([64, H], F32, tag="bpb", bufs=1)
                nc.gpsimd.dma_start(bp_bc[:], bcast(bp, 64))
                for c in range(H // GCHUNK):
                    ps = gps.tile([64, GCHUNK], F32, tag="g")
                    cs = slice(c * GCHUNK, (c + 1) * GCHUNK)
                    for k in range(KD):
                        nc.tensor.matmul(ps[:], hs_sb[k][:], wp_sb[k][:, cs],
                                         start=(k == 0), stop=(k == KD - 1))
                    nc.vector.tensor_add(hx_sb[:, cs], ps[:], bp_bc[:, cs])

                # XW = x @ W_ih.T + (b_ih + b_hh), rows r = t*64+b
                bg_bc = ph2.tile([128, G4], F32)
                nc.gpsimd.dma_start(bg_bc[:], bcast(bg, 128))
                wih_sb = [ph2.tile([128, G4], BF16, tag=f"wi{k}", name=f"wi{k}") for k in range(KD)]
                for k in range(KD):
                    nc.sync.dma_start(wih_sb[k][:], wih_hi[k])
                if terms == 3:
                    wil_sb = [ph2.tile([128, G4], BF16, tag=f"wil{k}", name=f"wil{k}")
                              for k in range(KD)]
                    for k in range(KD):
                        nc.sync.dma_start(wil_sb[k][:], wih_lo[k])
                nmm = (3 * KD) if terms == 3 else KD
                for mt in range(NMT):
                    ms = slice(mt * 128, (mt + 1) * 128)
                    xts = [ph2st.tile([128, 128], BF16, tag=f"xts{k}",
                                      name=f"xts{k}", bufs=2) for k in range(KD)]
                    for k in range(KD):
                        nc.sync.dma_start(xts[k][:], xT_hi[k][:, ms])
                    xw_terms = [(xts, wih_sb)]
                    if terms == 3:
                        xtls = [ph2st.tile([128, 128], BF16, tag=f"xtls{k}",
                                           name=f"xtls{k}", bufs=2)
                                for k in range(KD)]
                        for k in range(KD):
                            nc.sync.dma_start(xtls[k][:], xT_lo[k][:, ms])
                        xw_terms += [(xtls, wih_sb), (xts, wil_sb)]
                    for c in range(G4 // GCHUNK):
                        cs = slice(c * GCHUNK, (c + 1) * GCHUNK)
                        ps = gps.tile([128, GCHUNK], F32, tag="g")
                        j = 0
                        for lhs, rhs in xw_terms:
                            for k in range(KD):
                                nc.tensor.matmul(ps[:], lhs[k][:], rhs[k][:, cs],
                                                 start=(j == 0), stop=(j == nmm - 1))
                                j += 1
                        st = ph2st.tile([128, GCHUNK], F32, tag="xwst")
                        nc.vector.tensor_add(st[:], ps[:], bg_bc[:, cs])
                        nc.sync.dma_start(xw[ms, cs], st[:])

            # ---- loop-phase pools (reuse phase-2 address space) ----
            import contextlib
            _loop = contextlib.ExitStack()
            whlp = _loop.enter_context(tc.tile_pool(name="whlp", bufs=1))
            cellp = _loop.enter_context(tc.tile_pool(name="cell", bufs=1))
            xwbufp = _loop.enter_context(tc.tile_pool(name="xwbuf", bufs=1))
            gbufp = _loop.enter_context(tc.tile_pool(name="gbuf", bufs=2))
            wvsp = _loop.enter_context(tc.tile_pool(name="wvs", bufs=2))
            lstagep = _loop.enter_context(tc.tile_pool(name="lstage", bufs=2))

            if terms == 3:
                whl_sb = [whlp.tile([128, G4], BF16, tag=f"whhl{k}", name=f"whhl{k}")
                          for k in range(KH)]
                for k in range(KH):
                    nc.sync.dma_start(whl_sb[k][:], whh_lo[k])

            xwt = xwbufp.tile([128, G4], F32)   # xw(t) parity-packed
            cx_sb = cellp.tile([64, H], F32, tag="cx")
            nc.vector.memset(cx_sb[:], 0.0)
            sig_i = cellp.tile([64, H], F32, tag="si")   # later reused as i*g~
            tanh_g = cellp.tile([64, H], F32, tag="tg")  # later reused as tanh(cx)

            # ---- transpose h_t -> hxT ring slot (bf16 hi [+ lo]) ----
            def transpose_h(t):
                s = t % WIN
                for k in range(KH):
                    pst = tps.tile([128, 64], F32, tag="tp")
                    nc.tensor.transpose(pst[:], hx_sb[:, k * 128:(k + 1) * 128],
                                        ident[:])
                    dst = hxT[k][:, s * B:(s + 1) * B]
                    nc.vector.tensor_copy(dst, pst[:])
                    if terms == 3:
                        dlo = hxTlo[k][:, (t % 2) * B:(t % 2 + 1) * B]
                        nc.vector.tensor_sub(dlo, pst[:], dst)

            # ---- vocab job machinery ----
            wv_cur = {}

            def stream_wv(c):
                tiles = [wvsp.tile([128, VCHUNK], BF16, tag=f"wv{k}", name=f"wvs{k}")
                         for k in range(KH)]
                cs = slice(c * VCHUNK, (c + 1) * VCHUNK)
                for k in range(KH):
                    nc.sync.dma_start(tiles[k][:], wv[k][:, cs])
                return tiles

            def vocab_job(mt, c, wv_tiles):
                t0 = 2 * mt + 1          # rows mt*128.. use h_{t0}, h_{t0+1}
                s0, s1 = t0 % WIN, (t0 + 1) % WIN
                if s1 == s0 + 1:
                    ps = vps.tile([128, VCHUNK], F32, tag="vp")
                    lh = [hxT[k][:, s0 * B:(s0 + 2) * B] for k in range(KH)]
                    for k in range(KH):
                        nc.tensor.matmul(ps[:], lh[k], wv_tiles[k][:],
                                         start=(k == 0), stop=(k == KH - 1))
                    st = lstagep.tile([128, VCHUNK], F16, tag="lg")
                    nc.scalar.copy(st[:], ps[:])
                assert s1 == s0 + 1, "WIN=17 guarantees no ring straddle"
                nc.sync.dma_start(
                    logits[mt * 128:(mt + 1) * 128, c * VCHUNK:(c + 1) * VCHUNK],
                    st[:])

            vocab_q = []   # list of (mt, c) in stream order (c outer within batch)

            def emit_vocab(n):
                nonlocal wv_cur
                for _ in range(n):
                    if not vocab_q:
                        return
                    mt, c = vocab_q.pop(0)
                    if c not in wv_cur:
                        wv_cur = {c: stream_wv(c)}
                    vocab_job(mt, c, wv_cur[c])

            # ---- main recurrence ----
            transpose_h(0)
            # prefetch xw row-block for t=0 into parity half 0
            nc.sync.dma_start(xwt[0:64, :], xw[0:64, :])
            for t in range(L):
                par, nxt = (t % 2) * 64, ((t + 1) % 2) * 64
                if t + 1 < L:
                    nc.sync.dma_start(xwt[nxt:nxt + 64, :],
                                      xw[(t + 1) * 64:(t + 2) * 64, :])
                st_slot = t % WIN
                pairs = [(hxT[k][:, st_slot * B:(st_slot + 1) * B], whh_sb[k])
                         for k in range(KH)]
                if terms == 3:
                    pairs += [(hxT[k][:, st_slot * B:(st_slot + 1) * B], whl_sb[k])
                              for k in range(KH)]
                    pairs += [(hxTlo[k][:, (t % 2) * B:(t % 2 + 1) * B], whh_sb[k])
                              for k in range(KH)]
                # chain A = k-tiles 0-3 of each term, chain B = k-tiles 4-7
                gate_t = [p for i, p in enumerate(pairs) if i % KH < KH // 2] + \
                         [p for i, p in enumerate(pairs) if i % KH >= KH // 2]
                gparts = (sig_i, None, tanh_g, None)
                gfuncs = (SIGMOID, SIGMOID, TANH, SIGMOID)
                gb_o = [None, None]
                half_n = len(gate_t) // 2
                for c in range(G4 // GCHUNK):
                    cs = slice(c * GCHUNK, (c + 1) * GCHUNK)
                    gb = gbufp.tile([64, GCHUNK], F32, tag="gb")
                    if pack:
                        # two concurrent accumulation chains in array col-halves
                        ps = gps.tile([128, GCHUNK], F32, tag="g")
                        for j in range(half_n):
                            la, ra = gate_t[j]
                            lb, rb = gate_t[half_n + j]
                            nc.tensor.matmul(ps[0:64, :], la, ra[:, cs],
                                             start=(j == 0), stop=(j == half_n - 1),
                                             tile_position=(0, 0))
                            nc.tensor.matmul(ps[64:128, :], lb, rb[:, cs],
                                             start=(j == 0), stop=(j == half_n - 1),
                                             tile_position=(0, 64))
                        nc.vector.tensor_add(gb[:], ps[0:64, :],
                                             xwt[par:par + 64, cs])
                        nc.vector.tensor_add(gb[:], gb[:], ps[64:128, :])
                    else:
                        ps = gps.tile([64, GCHUNK], F32, tag="g")
                        for j, (lh, rh) in enumerate(gate_t):
                            nc.tensor.matmul(ps[:], lh, rh[:, cs],
                                             start=(j == 0),
                                             stop=(j == len(gate_t) - 1))
                        nc.vector.tensor_add(gb[:], ps[:], xwt[par:par + 64, cs])
                    part, half = gparts[c // 2], c % 2
                    hs = slice(half * GCHUNK, (half + 1) * GCHUNK)
                    if c // 2 == 1:    # f gate: fold sigmoid(f)*cx in place
                        nc.scalar.activation(gb[:], gb[:], SIGMOID)
                        nc.vector.tensor_mul(cx_sb[:, hs], cx_sb[:, hs], gb[:])
                    elif c // 2 == 3:  # o gate: sigmoid in place, kept in gb slot
                        nc.scalar.activation(gb[:], gb[:], SIGMOID)
                        gb_o[half] = gb
                    else:
                        nc.scalar.activation(part[:, hs], gb[:], gfuncs[c // 2])
                emit_vocab(2)
                # cell update
                nc.vector.tensor_mul(sig_i[:], sig_i[:], tanh_g[:])     # i*g~
                nc.vector.tensor_add(cx_sb[:], cx_sb[:], sig_i[:])
                nc.scalar.activation(tanh_g[:], cx_sb[:], TANH)          # tanh(cx)
                for half in range(2):
                    hs = slice(half * GCHUNK, (half + 1) * GCHUNK)
                    nc.vector.tensor_mul(hx_sb[:, hs], gb_o[half][:],
                                         tanh_g[:, hs])
                nc.sync.dma_start(hx_out[t], hx_sb[:])
                transpose_h(t + 1)
                emit_vocab(2)
                if t % 8 == 7:
                    b = t // 8
                    vocab_q += [(4 * b + i, c) for c in range(NVC)
                                for i in range(MT_BATCH)]
            while vocab_q:
                emit_vocab(1)
            _loop.close()

    return nc


_CACHED = {}


def get_compiled():
    if "nc" not in _CACHED:
        nc = build_nc()
        nc.compile()
        _CACHED["nc"] = nc
    return _CACHED["nc"]


def _bf16(x):
    return np.asarray(x, np.float32).astype(ml_dtypes.bfloat16)


def _split(x):
    hi = _bf16(x)
    lo = _bf16(np.asarray(x, np.float32) - hi.astype(np.float32))
    return hi, lo


def prep_in_maps(sent_inputs, hidden_state, W_ih, W_hh, b_ih, b_hh,
                 W_p, b_p, W_v, b_v):
    f32 = np.float32
    xT = np.ascontiguousarray(
        np.asarray(sent_inputs, f32).transpose(2, 1, 0)).reshape(KD, 128, R)
    xT_hi, xT_lo = _split(xT)
    hsT = np.ascontiguousarray(
        np.asarray(hidden_state, f32)[:, -1, :].T).reshape(KD, 128, B)
    wp = np.ascontiguousarray(np.asarray(W_p, f32).T).reshape(KD, 128, H)
    wih = np.ascontiguousarray(np.asarray(W_ih, f32).T).reshape(KD, 128, G4)
    wih_hi, wih_lo = _split(wih)
    whh = np.ascontiguousarray(np.asarray(W_hh, f32).T).reshape(KH, 128, G4)
    whh_hi, whh_lo = _split(whh)
    bg = (np.asarray(b_ih, f32) + np.asarray(b_hh, f32)).astype(f32)
    wvT = np.ascontiguousarray(np.asarray(W_v, f32).T)  # [H, V]
    base = dict(xT_hi=xT_hi, hsT=hsT, wp=wp, bp=np.asarray(b_p, f32),
                wih_hi=wih_hi, bg=bg)
    if TERMS == 3:
        base.update(xT_lo=xT_lo, wih_lo=wih_lo, whh_lo=whh_lo)
    in_maps = []
    for c in range(NCORES):
        m = dict(base)
        m["whh_hi"] = whh_hi
        sl = wvT[:, c * VS:(c + 1) * VS]
        m["wv"] = np.ascontiguousarray(sl).reshape(KH, 128, VS).astype(
            ml_dtypes.bfloat16)
        in_maps.append(m)
    return in_maps


def postprocess(results, W_v, b_v):
    f32 = np.float32
    logits = np.concatenate(
        [results[c]["logits"].astype(f32) for c in range(NCORES)], axis=1)
    logits += np.asarray(b_v, f32)[None, :]
    hx = results[0]["hx_out"]                      # [L, B, H] fp32
    # logits rows r = t*64+b -> [B, L, V]
    logits_blv = np.ascontiguousarray(
        logits.reshape(L, B, V).transpose(1, 0, 2))
    # predicts: top-k candidates from device logits, exact fp64 recompute
    K = 32
    flat = logits.reshape(R, V)
    cand = np.argpartition(-flat, K, axis=1)[:, :K]
    cand.sort(axis=1)
    Wv64 = np.asarray(W_v, np.float64)
    bv64 = np.asarray(b_v, np.float64)
    hx64 = hx.reshape(R, H).astype(np.float64)
    ex = np.einsum("rh,rkh->rk", hx64, Wv64[cand]) + bv64[cand]
    pick = cand[np.arange(R), np.argmax(ex, axis=1)]
    predicts = pick.reshape(L, B).T.astype(f32)    # [B, L]
    return logits_blv, predicts


def kernel(**inputs):
    nc = get_compiled()
    in_maps = prep_in_maps(**inputs)
    res = run_bass_kernel_spmd(nc, in_maps, core_ids=list(range(NCORES)))
    return postprocess(res.results, inputs["W_v"], inputs["b_v"])
